# revision 21
# baseline (speedup 1.0000x reference)
"""BSpline KAN layer (grid_size=5, spline_order=3) on 8 Trainium2 NeuronCores.

Strategy (data-parallel over batch):
  - Each core gets B_local = 512 rows of x, replicated weights.
  - Layout on-chip: in-dim on partitions (8 chunks of 128), batch on free dim.
  - Grid -> knots/reciprocals computed on-device per in-chunk column ([128,1]
    per-partition scalars for tensor_scalar / activation scale+bias).
  - Degree-1 bases as hat functions: b1[j] = min(relu(up), relu(down)), with
    up/down computed on the Scalar (ACT) engine as Relu(x*scale+bias).
  - Degrees 2/3 via Cox-de Boor with l/r factors from fused tensor_scalar
    ((x - g[j]) * recip) at DVE 4x mode, and the products/sums as j-stacked
    wide tensor_tensor ops ([128, 9*512]) to amortize instruction overhead.
  - Spline contraction as matmul with k-order j-major: k = j*1024 + i, so the
    j-stacked basis tiles are directly the matmul rhs. silu(x) @ base_weight.T
    is folded in as a 9th "basis" with base_weight as its weight block.
  - All 8 PSUM banks accumulate the 8 out-chunks across the whole contraction;
    epilogue adds res_scale * x and stores y[out, batch] (host transposes).
Precision: fp16 bases/weights, fp32 accumulation (emulated L2 rel err ~5e-4).
"""

import numpy as np

import concourse.bass as bass
from concourse import bacc
import concourse.mybir as mybir
import concourse.tile as tile
from concourse.alu_op_type import AluOpType
from concourse.bass_utils import run_bass_kernel_spmd

F32 = mybir.dt.float32
F16 = mybir.dt.float16
AF = mybir.ActivationFunctionType

IN_DIM = 1024
OUT_DIM = 1024
BATCH = 4096
N_CORES = 8
BL = BATCH // N_CORES        # 512 batch rows per core
NCH = IN_DIM // 128          # 8 in-dim chunks
NK = 12                      # knots per dim
EPS = 1e-8

LAST_PROFILE = {}

# engine for the stacked adds of the recursion (offload DVE)
B2_ADD_ENGINE = "gpsimd"
B3_ADD_ENGINE = "gpsimd"


def _build_nc():
    nc = bacc.Bacc("TRN2", target_bir_lowering=False)

    xt = nc.dram_tensor("xt", [IN_DIM, BL], F32, kind="ExternalInput")
    w = nc.dram_tensor("w", [9 * IN_DIM, OUT_DIM], F16, kind="ExternalInput")
    gsl = nc.dram_tensor("gsl", [128, NCH * (NK - 1)], F32, kind="ExternalInput")
    gst = nc.dram_tensor("gst", [128, NCH], F32, kind="ExternalInput")
    rs = nc.dram_tensor("rs", [1, 1], F32, kind="ExternalInput")
    y = nc.dram_tensor("y", [OUT_DIM, BL], F32, kind="ExternalOutput")

    with tile.TileContext(nc) as tc:
        with (
            tc.tile_pool(name="const", bufs=1) as cp,
            tc.tile_pool(name="xres", bufs=1) as xp,
            tc.tile_pool(name="small", bufs=2) as sp,
            tc.tile_pool(name="updn", bufs=2) as bp1,
            tc.tile_pool(name="lr2", bufs=2) as bp2,
            tc.tile_pool(name="lr3", bufs=2) as bp3,
            tc.tile_pool(name="wts", bufs=12) as wp,
            tc.tile_pool(name="yout", bufs=4) as yp,
            tc.tile_pool(name="psum", bufs=1, space="PSUM") as pp,
        ):
            # ---------------- grid preparation (once) ----------------
            gslT = cp.tile([128, NK - 1, NCH], F32)
            nc.gpsimd.dma_start(out=gslT[:, :, :],
                                in_=gsl[:, :].rearrange("p (k c) -> p k c", c=NCH))
            g3 = cp.tile([128, NK, NCH], F32)
            nc.gpsimd.dma_start(out=g3[:, 0, :], in_=gst[:, :])

            # softplus(v) = relu(v) + ln(1 + exp(-|v|))   (no softplus table
            # in the ACT func sets; exp/ln are in natural_log_exp_and_others)
            st3 = cp.tile([128, NK - 1, NCH], F32)
            spa = cp.tile([128, NK - 1, NCH], F32)
            nc.scalar.activation(spa[:, :, :], gslT[:, :, :], AF.Abs)
            nc.scalar.activation(spa[:, :, :], spa[:, :, :], AF.Exp, scale=-1.0)
            nc.scalar.activation(spa[:, :, :], spa[:, :, :], AF.Ln, bias=1.0)
            nc.scalar.activation(st3[:, :, :], gslT[:, :, :], AF.Relu)
            nc.vector.tensor_tensor(st3[:, :, :], st3[:, :, :], spa[:, :, :],
                                    AluOpType.add)
            for k in range(1, NK):
                nc.vector.tensor_tensor(g3[:, k, :], g3[:, k - 1, :],
                                        st3[:, k - 1, :], AluOpType.add)

            def recips(d, n):
                dt = cp.tile([128, n, NCH], F32, tag=f"d{d}")
                nc.vector.tensor_tensor(dt[:, :, :], g3[:, d:NK, :],
                                        g3[:, 0:NK - d, :], AluOpType.subtract)
                nc.vector.tensor_scalar_add(dt[:, :, :], dt[:, :, :], EPS)
                r = cp.tile([128, n, NCH], F32, tag=f"r{d}")
                nc.vector.reciprocal(r[:, :, :], dt[:, :, :])
                nr = cp.tile([128, n, NCH], F32, tag=f"nr{d}")
                nc.vector.tensor_scalar_mul(nr[:, :, :], r[:, :, :], -1.0)
                return r, nr

            R1, NR1 = recips(1, NK - 1)   # [128,8,11]
            R2, NR2 = recips(2, NK - 2)   # [128,8,10]
            R3, NR3 = recips(3, NK - 3)   # [128,8,9]

            # biases for the ACT hat ops
            BU = cp.tile([128, 10, NCH], F32)   # -g[j]*R1[j]
            nc.vector.scalar_tensor_tensor(BU[:, :, :], g3[:, 0:10, :], -1.0,
                                           R1[:, 0:10, :],
                                           AluOpType.mult, AluOpType.mult)
            BD = cp.tile([128, 10, NCH], F32)   # g[j+2]*R1[j+1]
            nc.vector.tensor_tensor(BD[:, :, :], g3[:, 2:12, :],
                                    R1[:, 1:11, :], AluOpType.mult)

            # biases for the ACT degree-3 factor ops
            BL3 = cp.tile([128, 8, NCH], F32)   # -g[j]*R3[j]
            nc.vector.scalar_tensor_tensor(BL3[:, :, :], g3[:, 0:8, :], -1.0,
                                           R3[:, 0:8, :],
                                           AluOpType.mult, AluOpType.mult)
            BR3 = cp.tile([128, 8, NCH], F32)   # g[j+4]*R3[j+1]
            nc.vector.tensor_tensor(BR3[:, :, :], g3[:, 4:12, :],
                                    R3[:, 1:9, :], AluOpType.mult)

            rs_t = cp.tile([128, 1], F32)
            nc.gpsimd.dma_start(out=rs_t[:, :], in_=rs[:].to_broadcast((128, 1)))

            # PSUM accumulators: one bank per out-chunk
            psum = [pp.tile([128, BL], F32, tag=f"ps{m}", name=f"ps{m}")
                    for m in range(NCH)]

            xc_tiles = []
            # ---------------- main loop over in-chunks ----------------
            for c in range(NCH):
                xc = xp.tile([128, BL], F32, tag=f"xc{c}")
                nc.sync.dma_start(out=xc[:, :], in_=xt[c * 128:(c + 1) * 128, :])
                xc_tiles.append(xc)

                x16 = sp.tile([128, BL], F16, tag="x16")
                nc.vector.tensor_scalar(x16[:, :], xc[:, :], 1.0,
                                        None, AluOpType.mult)
                sil = sp.tile([128, BL], F16, tag="sil")
                nc.scalar.activation(sil[:, :], x16[:, :], AF.Silu)

                # degree-1 hats on ACT + one wide min
                UP = bp1.tile([128, 10, BL], F16, tag="up")
                DN = bp1.tile([128, 10, BL], F16, tag="dn")
                for j in range(10):
                    nc.scalar.activation(UP[:, j, :], x16[:, :], AF.Relu,
                                         bias=BU[:, j, c:c+1], scale=R1[:, j, c:c+1])
                    nc.scalar.activation(DN[:, j, :], x16[:, :], AF.Relu,
                                         bias=BD[:, j, c:c+1], scale=NR1[:, j+1, c:c+1])
                nc.vector.tensor_tensor(UP[:, :, :], UP[:, :, :], DN[:, :, :],
                                        AluOpType.min)   # b1 := UP

                # degree 2
                L2 = bp2.tile([128, 9, BL], F16, tag="l2")
                R2t = bp2.tile([128, 9, BL], F16, tag="r2")
                for j in range(9):
                    nc.vector.tensor_scalar(L2[:, j, :], x16[:, :],
                                            g3[:, j, c:c+1], R2[:, j, c:c+1],
                                            AluOpType.subtract, AluOpType.mult)
                    nc.vector.tensor_scalar(R2t[:, j, :], x16[:, :],
                                            g3[:, j+3, c:c+1], NR2[:, j+1, c:c+1],
                                            AluOpType.subtract, AluOpType.mult)
                nc.vector.tensor_tensor(L2[:, :, :], L2[:, :, :],
                                        UP[:, 0:9, :], AluOpType.mult)
                nc.vector.tensor_tensor(R2t[:, :, :], R2t[:, :, :],
                                        UP[:, 1:10, :], AluOpType.mult)
                nc.vector.tensor_tensor(L2[:, :, :], L2[:, :, :], R2t[:, :, :],
                                        AluOpType.add)   # b2 := L2

                # degree 3 — l/r factors on ACT (Identity with scale+bias)
                L3 = bp3.tile([128, 8, BL], F16, tag="l3")
                R3t = bp3.tile([128, 8, BL], F16, tag="r3")
                for j in range(8):
                    if j < 3:
                        nc.vector.tensor_scalar(L3[:, j, :], x16[:, :],
                                                g3[:, j, c:c+1], R3[:, j, c:c+1],
                                                AluOpType.subtract, AluOpType.mult)
                    else:
                        nc.scalar.activation(L3[:, j, :], x16[:, :], AF.Identity,
                                             bias=BL3[:, j, c:c+1],
                                             scale=R3[:, j, c:c+1])
                    nc.scalar.activation(R3t[:, j, :], x16[:, :], AF.Identity,
                                         bias=BR3[:, j, c:c+1],
                                         scale=NR3[:, j+1, c:c+1])
                nc.vector.tensor_tensor(L3[:, :, :], L3[:, :, :],
                                        L2[:, 0:8, :], AluOpType.mult)
                nc.vector.tensor_tensor(R3t[:, :, :], R3t[:, :, :],
                                        L2[:, 1:9, :], AluOpType.mult)
                # b3 := L3 ; for the last chunk split the add into pieces
                # so the trailing matmuls can start incrementally
                if c < NCH - 1:
                    nc.vector.tensor_tensor(L3[:, :, :], L3[:, :, :],
                                            R3t[:, :, :], AluOpType.add)
                else:
                    for q in range(4):
                        nc.vector.tensor_tensor(L3[:, 2*q:2*q+2, :],
                                                L3[:, 2*q:2*q+2, :],
                                                R3t[:, 2*q:2*q+2, :],
                                                AluOpType.add)

                # matmuls: 9 weight blocks (8 spline j's + silu/base_weight)
                wts = []
                for j in range(9):
                    kc = j * NCH + c
                    wt = wp.tile([128, OUT_DIM], F16, tag="wt", name=f"wt{c}_{j}")
                    nc.sync.dma_start(out=wt[:, :],
                                      in_=w[kc * 128:(kc + 1) * 128, :])
                    wts.append(wt)

                def rhs_of(j):
                    return L3[:, j, :] if j < 8 else sil[:, :]

                if c < NCH - 1:
                    for j in range(9):
                        for m in range(NCH):
                            nc.tensor.matmul(psum[m][:, :],
                                             lhsT=wts[j][:, m * 128:(m + 1) * 128],
                                             rhs=rhs_of(j),
                                             start=(c == 0 and j == 0),
                                             stop=False,
                                             skip_group_check=True)
                else:
                    # spline mms j-outer (start as each add-piece lands), then
                    # the silu block last with per-bank epilogue interleaved
                    for j in range(8):
                        for m in range(NCH):
                            nc.tensor.matmul(psum[m][:, :],
                                             lhsT=wts[j][:, m * 128:(m + 1) * 128],
                                             rhs=rhs_of(j),
                                             start=False, stop=False,
                                             skip_group_check=True)
                    for m in range(NCH):
                        nc.tensor.matmul(psum[m][:, :],
                                         lhsT=wts[8][:, m * 128:(m + 1) * 128],
                                         rhs=rhs_of(8),
                                         start=False, stop=True,
                                         skip_group_check=True)
                        yt = yp.tile([128, BL], F32, tag="yt", name=f"yt{m}")
                        nc.vector.scalar_tensor_tensor(yt[:, :],
                                                       xc_tiles[m][:, :],
                                                       rs_t[:, :], psum[m][:, :],
                                                       AluOpType.mult,
                                                       AluOpType.add)
                        nc.sync.dma_start(out=y[m * 128:(m + 1) * 128, :],
                                          in_=yt[:, :])

    nc.compile()
    return nc


_NC_CACHE = None


def kernel(x, coeffs, base_weight, grid_steps_log, grid_start, res_scale,
           _trace=False):
    global _NC_CACHE, LAST_PROFILE

    x = np.asarray(x, dtype=np.float32)
    coeffs = np.asarray(coeffs, dtype=np.float32)
    base_weight = np.asarray(base_weight, dtype=np.float32)
    grid_steps_log = np.asarray(grid_steps_log, dtype=np.float32)
    grid_start = np.asarray(grid_start, dtype=np.float32)
    res_scale = np.asarray(res_scale, dtype=np.float32)

    # ---- host-side layout prep (pure reshape/transpose/dtype) ----
    # weights, k-order j-major: k = j*IN_DIM + i ; block j=8 is base_weight.T
    wj = coeffs.reshape(OUT_DIM, IN_DIM, 8).transpose(2, 1, 0)    # [8, in, out]
    big_w = np.concatenate([wj, base_weight.T[None]], axis=0)     # [9, in, out]
    big_w = np.ascontiguousarray(big_w.reshape(9 * IN_DIM, OUT_DIM),
                                 dtype=np.float16)

    xT = np.ascontiguousarray(x.T)                                # [in, B]
    # grid params: partition = in-dim within chunk, free = (chunk, knot)
    gsl_r = np.ascontiguousarray(
        grid_steps_log.reshape(NCH, 128, NK - 1).transpose(1, 2, 0)
        .reshape(128, (NK - 1) * NCH))
    gst_r = np.ascontiguousarray(
        grid_start.reshape(NCH, 128).T)                           # [128, 8]
    rs_r = res_scale.reshape(1, 1)

    if _NC_CACHE is None:
        _NC_CACHE = _build_nc()
    nc = _NC_CACHE

    in_maps = []
    for c in range(N_CORES):
        in_maps.append({
            "xt": np.ascontiguousarray(xT[:, c * BL:(c + 1) * BL]),
            "w": big_w,
            "gsl": gsl_r,
            "gst": gst_r,
            "rs": rs_r,
        })

    res = run_bass_kernel_spmd(nc, in_maps, core_ids=list(range(N_CORES)),
                               trace=_trace)
    LAST_PROFILE = {
        "exec_time_ns": res.exec_time_ns,
        "mean_exec_time_ns": res.mean_exec_time_ns,
        "max_exec_time_core_id": res.max_exec_time_core_id,
        "profile_json": res.profile_json,
        "instructions_and_trace": res.instructions_and_trace,
    }

    out = np.concatenate([r["y"].T for r in res.results], axis=0)  # [B, out]
    return np.ascontiguousarray(out.astype(np.float32))


# revision 22
# speedup vs baseline: 1.0078x; 1.0078x over previous
"""BSpline KAN layer (grid_size=5, spline_order=3) on 8 Trainium2 NeuronCores.

Strategy (data-parallel over batch):
  - Each core gets B_local = 512 rows of x, replicated weights.
  - Layout on-chip: in-dim on partitions (8 chunks of 128), batch on free dim.
  - Grid -> knots/reciprocals computed on-device per in-chunk column ([128,1]
    per-partition scalars for tensor_scalar / activation scale+bias).
  - Degree-1 bases as hat functions: b1[j] = min(relu(up), relu(down)), with
    up/down computed on the Scalar (ACT) engine as Relu(x*scale+bias).
  - Degrees 2/3 via Cox-de Boor with l/r factors from fused tensor_scalar
    ((x - g[j]) * recip) at DVE 4x mode, and the products/sums as j-stacked
    wide tensor_tensor ops ([128, 9*512]) to amortize instruction overhead.
  - Spline contraction as matmul with k-order j-major: k = j*1024 + i, so the
    j-stacked basis tiles are directly the matmul rhs. silu(x) @ base_weight.T
    is folded in as a 9th "basis" with base_weight as its weight block.
  - All 8 PSUM banks accumulate the 8 out-chunks across the whole contraction;
    epilogue adds res_scale * x and stores y[out, batch] (host transposes).
Precision: fp16 bases/weights, fp32 accumulation (emulated L2 rel err ~5e-4).
"""

import numpy as np

import concourse.bass as bass
from concourse import bacc
import concourse.mybir as mybir
import concourse.tile as tile
from concourse.alu_op_type import AluOpType
from concourse.bass_utils import run_bass_kernel_spmd

F32 = mybir.dt.float32
F16 = mybir.dt.float16
AF = mybir.ActivationFunctionType

IN_DIM = 1024
OUT_DIM = 1024
BATCH = 4096
N_CORES = 8
BL = BATCH // N_CORES        # 512 batch rows per core
NCH = IN_DIM // 128          # 8 in-dim chunks
NK = 12                      # knots per dim
EPS = 1e-8

LAST_PROFILE = {}

# engine for the stacked adds of the recursion (offload DVE)
B2_ADD_ENGINE = "gpsimd"
B3_ADD_ENGINE = "gpsimd"


def _build_nc():
    nc = bacc.Bacc("TRN2", target_bir_lowering=False)

    xt = nc.dram_tensor("xt", [IN_DIM, BL], F32, kind="ExternalInput")
    w = nc.dram_tensor("w", [9 * IN_DIM, OUT_DIM], F16, kind="ExternalInput")
    gsl = nc.dram_tensor("gsl", [128, NCH * (NK - 1)], F32, kind="ExternalInput")
    gst = nc.dram_tensor("gst", [128, NCH], F32, kind="ExternalInput")
    rs = nc.dram_tensor("rs", [1, 1], F32, kind="ExternalInput")
    y = nc.dram_tensor("y", [OUT_DIM, BL], F32, kind="ExternalOutput")

    with tile.TileContext(nc) as tc:
        with (
            tc.tile_pool(name="const", bufs=1) as cp,
            tc.tile_pool(name="xres", bufs=1) as xp,
            tc.tile_pool(name="small", bufs=4) as sp,
            tc.tile_pool(name="updn", bufs=2) as bp1,
            tc.tile_pool(name="lr2", bufs=2) as bp2,
            tc.tile_pool(name="lr3", bufs=3) as bp3,
            tc.tile_pool(name="wts", bufs=12) as wp,
            tc.tile_pool(name="yout", bufs=4) as yp,
            tc.tile_pool(name="psum", bufs=1, space="PSUM") as pp,
        ):
            # ---------------- grid preparation (once) ----------------
            gslT = cp.tile([128, NK - 1, NCH], F32)
            nc.gpsimd.dma_start(out=gslT[:, :, :],
                                in_=gsl[:, :].rearrange("p (k c) -> p k c", c=NCH))
            g3 = cp.tile([128, NK, NCH], F32)
            nc.gpsimd.dma_start(out=g3[:, 0, :], in_=gst[:, :])

            # softplus(v) = relu(v) + ln(1 + exp(-|v|))   (no softplus table
            # in the ACT func sets; exp/ln are in natural_log_exp_and_others)
            st3 = cp.tile([128, NK - 1, NCH], F32)
            spa = cp.tile([128, NK - 1, NCH], F32)
            nc.scalar.activation(spa[:, :, :], gslT[:, :, :], AF.Abs)
            nc.scalar.activation(spa[:, :, :], spa[:, :, :], AF.Exp, scale=-1.0)
            nc.scalar.activation(spa[:, :, :], spa[:, :, :], AF.Ln, bias=1.0)
            nc.scalar.activation(st3[:, :, :], gslT[:, :, :], AF.Relu)
            nc.vector.tensor_tensor(st3[:, :, :], st3[:, :, :], spa[:, :, :],
                                    AluOpType.add)
            for k in range(1, NK):
                nc.vector.tensor_tensor(g3[:, k, :], g3[:, k - 1, :],
                                        st3[:, k - 1, :], AluOpType.add)

            def recips(d, n):
                dt = cp.tile([128, n, NCH], F32, tag=f"d{d}")
                nc.vector.tensor_tensor(dt[:, :, :], g3[:, d:NK, :],
                                        g3[:, 0:NK - d, :], AluOpType.subtract)
                nc.vector.tensor_scalar_add(dt[:, :, :], dt[:, :, :], EPS)
                r = cp.tile([128, n, NCH], F32, tag=f"r{d}")
                nc.vector.reciprocal(r[:, :, :], dt[:, :, :])
                nr = cp.tile([128, n, NCH], F32, tag=f"nr{d}")
                nc.vector.tensor_scalar_mul(nr[:, :, :], r[:, :, :], -1.0)
                return r, nr

            R1, NR1 = recips(1, NK - 1)   # [128,8,11]
            R2, NR2 = recips(2, NK - 2)   # [128,8,10]
            R3, NR3 = recips(3, NK - 3)   # [128,8,9]

            # biases for the ACT hat ops
            BU = cp.tile([128, 10, NCH], F32)   # -g[j]*R1[j]
            nc.vector.scalar_tensor_tensor(BU[:, :, :], g3[:, 0:10, :], -1.0,
                                           R1[:, 0:10, :],
                                           AluOpType.mult, AluOpType.mult)
            BD = cp.tile([128, 10, NCH], F32)   # g[j+2]*R1[j+1]
            nc.vector.tensor_tensor(BD[:, :, :], g3[:, 2:12, :],
                                    R1[:, 1:11, :], AluOpType.mult)

            # biases for the ACT degree-3 factor ops
            BL3 = cp.tile([128, 8, NCH], F32)   # -g[j]*R3[j]
            nc.vector.scalar_tensor_tensor(BL3[:, :, :], g3[:, 0:8, :], -1.0,
                                           R3[:, 0:8, :],
                                           AluOpType.mult, AluOpType.mult)
            BR3 = cp.tile([128, 8, NCH], F32)   # g[j+4]*R3[j+1]
            nc.vector.tensor_tensor(BR3[:, :, :], g3[:, 4:12, :],
                                    R3[:, 1:9, :], AluOpType.mult)

            rs_t = cp.tile([128, 1], F32)
            nc.gpsimd.dma_start(out=rs_t[:, :], in_=rs[:].to_broadcast((128, 1)))

            # PSUM accumulators: one bank per out-chunk
            psum = [pp.tile([128, BL], F32, tag=f"ps{m}", name=f"ps{m}")
                    for m in range(NCH)]

            xc_tiles = []
            # ---------------- main loop over in-chunks ----------------
            for c in range(NCH):
                xc = xp.tile([128, BL], F32, tag=f"xc{c}")
                nc.sync.dma_start(out=xc[:, :], in_=xt[c * 128:(c + 1) * 128, :])
                xc_tiles.append(xc)

                x16 = sp.tile([128, BL], F16, tag="x16")
                nc.vector.tensor_scalar(x16[:, :], xc[:, :], 1.0,
                                        None, AluOpType.mult)
                sil = sp.tile([128, BL], F16, tag="sil")
                nc.scalar.activation(sil[:, :], x16[:, :], AF.Silu)

                # degree-1 hats on ACT + one wide min
                UP = bp1.tile([128, 10, BL], F16, tag="up")
                DN = bp1.tile([128, 10, BL], F16, tag="dn")
                for j in range(10):
                    nc.scalar.activation(UP[:, j, :], x16[:, :], AF.Relu,
                                         bias=BU[:, j, c:c+1], scale=R1[:, j, c:c+1])
                    nc.scalar.activation(DN[:, j, :], x16[:, :], AF.Relu,
                                         bias=BD[:, j, c:c+1], scale=NR1[:, j+1, c:c+1])
                nc.vector.tensor_tensor(UP[:, :, :], UP[:, :, :], DN[:, :, :],
                                        AluOpType.min)   # b1 := UP

                # degree 2
                L2 = bp2.tile([128, 9, BL], F16, tag="l2")
                R2t = bp2.tile([128, 9, BL], F16, tag="r2")
                for j in range(9):
                    nc.vector.tensor_scalar(L2[:, j, :], x16[:, :],
                                            g3[:, j, c:c+1], R2[:, j, c:c+1],
                                            AluOpType.subtract, AluOpType.mult)
                    nc.vector.tensor_scalar(R2t[:, j, :], x16[:, :],
                                            g3[:, j+3, c:c+1], NR2[:, j+1, c:c+1],
                                            AluOpType.subtract, AluOpType.mult)
                nc.vector.tensor_tensor(L2[:, :, :], L2[:, :, :],
                                        UP[:, 0:9, :], AluOpType.mult)
                nc.vector.tensor_tensor(R2t[:, :, :], R2t[:, :, :],
                                        UP[:, 1:10, :], AluOpType.mult)
                nc.vector.tensor_tensor(L2[:, :, :], L2[:, :, :], R2t[:, :, :],
                                        AluOpType.add)   # b2 := L2

                # degree 3 — l/r factors on ACT (Identity with scale+bias)
                L3 = bp3.tile([128, 8, BL], F16, tag="l3")
                R3t = bp3.tile([128, 8, BL], F16, tag="r3")
                for j in range(8):
                    if j < 3:
                        nc.vector.tensor_scalar(L3[:, j, :], x16[:, :],
                                                g3[:, j, c:c+1], R3[:, j, c:c+1],
                                                AluOpType.subtract, AluOpType.mult)
                    else:
                        nc.scalar.activation(L3[:, j, :], x16[:, :], AF.Identity,
                                             bias=BL3[:, j, c:c+1],
                                             scale=R3[:, j, c:c+1])
                    nc.scalar.activation(R3t[:, j, :], x16[:, :], AF.Identity,
                                         bias=BR3[:, j, c:c+1],
                                         scale=NR3[:, j+1, c:c+1])
                nc.vector.tensor_tensor(L3[:, :, :], L3[:, :, :],
                                        L2[:, 0:8, :], AluOpType.mult)
                nc.vector.tensor_tensor(R3t[:, :, :], R3t[:, :, :],
                                        L2[:, 1:9, :], AluOpType.mult)
                nc.vector.tensor_tensor(L3[:, :, :], L3[:, :, :],
                                        R3t[:, :, :], AluOpType.add)  # b3

                # matmuls: 9 weight blocks (8 spline j's + silu/base_weight)
                wts = []
                for j in range(9):
                    kc = j * NCH + c
                    wt = wp.tile([128, OUT_DIM], F16, tag="wt", name=f"wt{c}_{j}")
                    nc.sync.dma_start(out=wt[:, :],
                                      in_=w[kc * 128:(kc + 1) * 128, :])
                    wts.append(wt)

                def rhs_of(j):
                    return L3[:, j, :] if j < 8 else sil[:, :]

                if c < NCH - 1:
                    for j in range(9):
                        for m in range(NCH):
                            nc.tensor.matmul(psum[m][:, :],
                                             lhsT=wts[j][:, m * 128:(m + 1) * 128],
                                             rhs=rhs_of(j),
                                             start=(c == 0 and j == 0),
                                             stop=False,
                                             skip_group_check=True)
                else:
                    # last chunk: m-outer so each PSUM bank finishes early and
                    # its epilogue overlaps the remaining matmuls
                    for m in range(NCH):
                        for j in range(9):
                            nc.tensor.matmul(psum[m][:, :],
                                             lhsT=wts[j][:, m * 128:(m + 1) * 128],
                                             rhs=rhs_of(j),
                                             start=False,
                                             stop=(j == 8),
                                             skip_group_check=True)
                        yt = yp.tile([128, BL], F32, tag="yt", name=f"yt{m}")
                        nc.vector.scalar_tensor_tensor(yt[:, :],
                                                       xc_tiles[m][:, :],
                                                       rs_t[:, :], psum[m][:, :],
                                                       AluOpType.mult,
                                                       AluOpType.add)
                        nc.sync.dma_start(out=y[m * 128:(m + 1) * 128, :],
                                          in_=yt[:, :])

    nc.compile()
    return nc


_NC_CACHE = None


def kernel(x, coeffs, base_weight, grid_steps_log, grid_start, res_scale,
           _trace=False):
    global _NC_CACHE, LAST_PROFILE

    x = np.asarray(x, dtype=np.float32)
    coeffs = np.asarray(coeffs, dtype=np.float32)
    base_weight = np.asarray(base_weight, dtype=np.float32)
    grid_steps_log = np.asarray(grid_steps_log, dtype=np.float32)
    grid_start = np.asarray(grid_start, dtype=np.float32)
    res_scale = np.asarray(res_scale, dtype=np.float32)

    # ---- host-side layout prep (pure reshape/transpose/dtype) ----
    # weights, k-order j-major: k = j*IN_DIM + i ; block j=8 is base_weight.T
    wj = coeffs.reshape(OUT_DIM, IN_DIM, 8).transpose(2, 1, 0)    # [8, in, out]
    big_w = np.concatenate([wj, base_weight.T[None]], axis=0)     # [9, in, out]
    big_w = np.ascontiguousarray(big_w.reshape(9 * IN_DIM, OUT_DIM),
                                 dtype=np.float16)

    xT = np.ascontiguousarray(x.T)                                # [in, B]
    # grid params: partition = in-dim within chunk, free = (chunk, knot)
    gsl_r = np.ascontiguousarray(
        grid_steps_log.reshape(NCH, 128, NK - 1).transpose(1, 2, 0)
        .reshape(128, (NK - 1) * NCH))
    gst_r = np.ascontiguousarray(
        grid_start.reshape(NCH, 128).T)                           # [128, 8]
    rs_r = res_scale.reshape(1, 1)

    if _NC_CACHE is None:
        _NC_CACHE = _build_nc()
    nc = _NC_CACHE

    in_maps = []
    for c in range(N_CORES):
        in_maps.append({
            "xt": np.ascontiguousarray(xT[:, c * BL:(c + 1) * BL]),
            "w": big_w,
            "gsl": gsl_r,
            "gst": gst_r,
            "rs": rs_r,
        })

    res = run_bass_kernel_spmd(nc, in_maps, core_ids=list(range(N_CORES)),
                               trace=_trace)
    LAST_PROFILE = {
        "exec_time_ns": res.exec_time_ns,
        "mean_exec_time_ns": res.mean_exec_time_ns,
        "max_exec_time_core_id": res.max_exec_time_core_id,
        "profile_json": res.profile_json,
        "instructions_and_trace": res.instructions_and_trace,
    }

    out = np.concatenate([r["y"].T for r in res.results], axis=0)  # [B, out]
    return np.ascontiguousarray(out.astype(np.float32))


# revision 23
# speedup vs baseline: 1.0087x; 1.0008x over previous
"""BSpline KAN layer (grid_size=5, spline_order=3) on 8 Trainium2 NeuronCores.

Strategy (data-parallel over batch):
  - Each core gets B_local = 512 rows of x, replicated weights.
  - Layout on-chip: in-dim on partitions (8 chunks of 128), batch on free dim.
  - Grid -> knots/reciprocals computed on-device per in-chunk column ([128,1]
    per-partition scalars for tensor_scalar / activation scale+bias).
  - Degree-1 bases as hat functions: b1[j] = min(relu(up), relu(down)), with
    up/down computed on the Scalar (ACT) engine as Relu(x*scale+bias).
  - Degrees 2/3 via Cox-de Boor with l/r factors from fused tensor_scalar
    ((x - g[j]) * recip) at DVE 4x mode, and the products/sums as j-stacked
    wide tensor_tensor ops ([128, 9*512]) to amortize instruction overhead.
  - Spline contraction as matmul with k-order j-major: k = j*1024 + i, so the
    j-stacked basis tiles are directly the matmul rhs. silu(x) @ base_weight.T
    is folded in as a 9th "basis" with base_weight as its weight block.
  - All 8 PSUM banks accumulate the 8 out-chunks across the whole contraction;
    epilogue adds res_scale * x and stores y[out, batch] (host transposes).
Precision: fp16 bases/weights, fp32 accumulation (emulated L2 rel err ~5e-4).
"""

import numpy as np

import concourse.bass as bass
from concourse import bacc
import concourse.mybir as mybir
import concourse.tile as tile
from concourse.alu_op_type import AluOpType
from concourse.bass_utils import run_bass_kernel_spmd

F32 = mybir.dt.float32
F16 = mybir.dt.float16
AF = mybir.ActivationFunctionType

IN_DIM = 1024
OUT_DIM = 1024
BATCH = 4096
N_CORES = 8
BL = BATCH // N_CORES        # 512 batch rows per core
NCH = IN_DIM // 128          # 8 in-dim chunks
NK = 12                      # knots per dim
EPS = 1e-8

LAST_PROFILE = {}

# engine for the stacked adds of the recursion (offload DVE)
B2_ADD_ENGINE = "gpsimd"
B3_ADD_ENGINE = "gpsimd"


def _build_nc():
    nc = bacc.Bacc("TRN2", target_bir_lowering=False)

    xt = nc.dram_tensor("xt", [IN_DIM, BL], F32, kind="ExternalInput")
    w = nc.dram_tensor("w", [9 * IN_DIM, OUT_DIM], F16, kind="ExternalInput")
    gsl = nc.dram_tensor("gsl", [128, NCH * (NK - 1)], F32, kind="ExternalInput")
    gst = nc.dram_tensor("gst", [128, NCH], F32, kind="ExternalInput")
    rs = nc.dram_tensor("rs", [1, 1], F32, kind="ExternalInput")
    y = nc.dram_tensor("y", [OUT_DIM, BL], F32, kind="ExternalOutput")

    with tile.TileContext(nc) as tc:
        with (
            tc.tile_pool(name="const", bufs=1) as cp,
            tc.tile_pool(name="xres", bufs=1) as xp,
            tc.tile_pool(name="small", bufs=4) as sp,
            tc.tile_pool(name="updn", bufs=2) as bp1,
            tc.tile_pool(name="lr2", bufs=2) as bp2,
            tc.tile_pool(name="lr3", bufs=3) as bp3,
            tc.tile_pool(name="wts", bufs=12) as wp,
            tc.tile_pool(name="yout", bufs=4) as yp,
            tc.tile_pool(name="psum", bufs=1, space="PSUM") as pp,
        ):
            # ---------------- grid preparation (once) ----------------
            gslT = cp.tile([128, NK - 1, NCH], F32)
            nc.gpsimd.dma_start(out=gslT[:, :, :],
                                in_=gsl[:, :].rearrange("p (k c) -> p k c", c=NCH))
            g3 = cp.tile([128, NK, NCH], F32)
            nc.gpsimd.dma_start(out=g3[:, 0, :], in_=gst[:, :])

            # softplus(v) = relu(v) + ln(1 + exp(-|v|))   (no softplus table
            # in the ACT func sets; exp/ln are in natural_log_exp_and_others)
            st3 = cp.tile([128, NK - 1, NCH], F32)
            spa = cp.tile([128, NK - 1, NCH], F32)
            nc.scalar.activation(spa[:, :, :], gslT[:, :, :], AF.Abs)
            nc.scalar.activation(spa[:, :, :], spa[:, :, :], AF.Exp, scale=-1.0)
            nc.scalar.activation(spa[:, :, :], spa[:, :, :], AF.Ln, bias=1.0)
            nc.scalar.activation(st3[:, :, :], gslT[:, :, :], AF.Relu)
            nc.vector.tensor_tensor(st3[:, :, :], st3[:, :, :], spa[:, :, :],
                                    AluOpType.add)
            for k in range(1, NK):
                nc.vector.tensor_tensor(g3[:, k, :], g3[:, k - 1, :],
                                        st3[:, k - 1, :], AluOpType.add)

            def recips(d, n):
                dt = cp.tile([128, n, NCH], F32, tag=f"d{d}")
                nc.vector.tensor_tensor(dt[:, :, :], g3[:, d:NK, :],
                                        g3[:, 0:NK - d, :], AluOpType.subtract)
                nc.vector.tensor_scalar_add(dt[:, :, :], dt[:, :, :], EPS)
                r = cp.tile([128, n, NCH], F32, tag=f"r{d}")
                nc.vector.reciprocal(r[:, :, :], dt[:, :, :])
                nr = cp.tile([128, n, NCH], F32, tag=f"nr{d}")
                nc.vector.tensor_scalar_mul(nr[:, :, :], r[:, :, :], -1.0)
                return r, nr

            R1, NR1 = recips(1, NK - 1)   # [128,8,11]
            R2, NR2 = recips(2, NK - 2)   # [128,8,10]
            R3, NR3 = recips(3, NK - 3)   # [128,8,9]

            # biases for the ACT hat ops
            BU = cp.tile([128, 10, NCH], F32)   # -g[j]*R1[j]
            nc.vector.scalar_tensor_tensor(BU[:, :, :], g3[:, 0:10, :], -1.0,
                                           R1[:, 0:10, :],
                                           AluOpType.mult, AluOpType.mult)
            BD = cp.tile([128, 10, NCH], F32)   # g[j+2]*R1[j+1]
            nc.vector.tensor_tensor(BD[:, :, :], g3[:, 2:12, :],
                                    R1[:, 1:11, :], AluOpType.mult)

            # biases for the ACT degree-3 factor ops
            BL3 = cp.tile([128, 8, NCH], F32)   # -g[j]*R3[j]
            nc.vector.scalar_tensor_tensor(BL3[:, :, :], g3[:, 0:8, :], -1.0,
                                           R3[:, 0:8, :],
                                           AluOpType.mult, AluOpType.mult)
            BR3 = cp.tile([128, 8, NCH], F32)   # g[j+4]*R3[j+1]
            nc.vector.tensor_tensor(BR3[:, :, :], g3[:, 4:12, :],
                                    R3[:, 1:9, :], AluOpType.mult)

            rs_t = cp.tile([128, 1], F32)
            nc.gpsimd.dma_start(out=rs_t[:, :], in_=rs[:].to_broadcast((128, 1)))

            # PSUM accumulators: one bank per out-chunk
            psum = [pp.tile([128, BL], F32, tag=f"ps{m}", name=f"ps{m}")
                    for m in range(NCH)]

            xc_tiles = []
            # ---------------- main loop over in-chunks ----------------
            for c in range(NCH):
                xc = xp.tile([128, BL], F32, tag=f"xc{c}")
                nc.sync.dma_start(out=xc[:, :], in_=xt[c * 128:(c + 1) * 128, :])
                xc_tiles.append(xc)

                x16 = sp.tile([128, BL], F16, tag="x16")
                nc.vector.tensor_scalar(x16[:, :], xc[:, :], 1.0,
                                        None, AluOpType.mult)
                # degree-1 hats on ACT; the independent DVE tensor_scalar
                # factor ops are emitted FIRST so the in-order DVE queue has
                # ready work while the 20 ACT hat ops complete (the min below
                # blocks the DVE FIFO until the hats land)
                UP = bp1.tile([128, 10, BL], F16, tag="up")
                DN = bp1.tile([128, 10, BL], F16, tag="dn")
                for j in range(10):
                    nc.scalar.activation(UP[:, j, :], x16[:, :], AF.Relu,
                                         bias=BU[:, j, c:c+1], scale=R1[:, j, c:c+1])
                    nc.scalar.activation(DN[:, j, :], x16[:, :], AF.Relu,
                                         bias=BD[:, j, c:c+1], scale=NR1[:, j+1, c:c+1])

                L2 = bp2.tile([128, 9, BL], F16, tag="l2")
                R2t = bp2.tile([128, 9, BL], F16, tag="r2")
                L3 = bp3.tile([128, 8, BL], F16, tag="l3")
                R3t = bp3.tile([128, 8, BL], F16, tag="r3")
                for j in range(9):
                    nc.vector.tensor_scalar(L2[:, j, :], x16[:, :],
                                            g3[:, j, c:c+1], R2[:, j, c:c+1],
                                            AluOpType.subtract, AluOpType.mult)
                    nc.vector.tensor_scalar(R2t[:, j, :], x16[:, :],
                                            g3[:, j+3, c:c+1], NR2[:, j+1, c:c+1],
                                            AluOpType.subtract, AluOpType.mult)
                for j in range(3):
                    nc.vector.tensor_scalar(L3[:, j, :], x16[:, :],
                                            g3[:, j, c:c+1], R3[:, j, c:c+1],
                                            AluOpType.subtract, AluOpType.mult)
                for j in range(8):
                    if j >= 3:
                        nc.scalar.activation(L3[:, j, :], x16[:, :], AF.Identity,
                                             bias=BL3[:, j, c:c+1],
                                             scale=R3[:, j, c:c+1])
                    nc.scalar.activation(R3t[:, j, :], x16[:, :], AF.Identity,
                                         bias=BR3[:, j, c:c+1],
                                         scale=NR3[:, j+1, c:c+1])
                sil = sp.tile([128, BL], F16, tag="sil")
                nc.scalar.activation(sil[:, :], x16[:, :], AF.Silu)

                nc.vector.tensor_tensor(UP[:, :, :], UP[:, :, :], DN[:, :, :],
                                        AluOpType.min)   # b1 := UP
                nc.vector.tensor_tensor(L2[:, :, :], L2[:, :, :],
                                        UP[:, 0:9, :], AluOpType.mult)
                nc.vector.tensor_tensor(R2t[:, :, :], R2t[:, :, :],
                                        UP[:, 1:10, :], AluOpType.mult)
                nc.vector.tensor_tensor(L2[:, :, :], L2[:, :, :], R2t[:, :, :],
                                        AluOpType.add)   # b2 := L2
                nc.vector.tensor_tensor(L3[:, :, :], L3[:, :, :],
                                        L2[:, 0:8, :], AluOpType.mult)
                nc.vector.tensor_tensor(R3t[:, :, :], R3t[:, :, :],
                                        L2[:, 1:9, :], AluOpType.mult)
                nc.vector.tensor_tensor(L3[:, :, :], L3[:, :, :],
                                        R3t[:, :, :], AluOpType.add)  # b3

                # matmuls: 9 weight blocks (8 spline j's + silu/base_weight)
                wts = []
                for j in range(9):
                    kc = j * NCH + c
                    wt = wp.tile([128, OUT_DIM], F16, tag="wt", name=f"wt{c}_{j}")
                    nc.sync.dma_start(out=wt[:, :],
                                      in_=w[kc * 128:(kc + 1) * 128, :])
                    wts.append(wt)

                def rhs_of(j):
                    return L3[:, j, :] if j < 8 else sil[:, :]

                if c < NCH - 1:
                    for j in range(9):
                        for m in range(NCH):
                            nc.tensor.matmul(psum[m][:, :],
                                             lhsT=wts[j][:, m * 128:(m + 1) * 128],
                                             rhs=rhs_of(j),
                                             start=(c == 0 and j == 0),
                                             stop=False,
                                             skip_group_check=True)
                else:
                    # last chunk: m-outer so each PSUM bank finishes early and
                    # its epilogue overlaps the remaining matmuls
                    for m in range(NCH):
                        for j in range(9):
                            nc.tensor.matmul(psum[m][:, :],
                                             lhsT=wts[j][:, m * 128:(m + 1) * 128],
                                             rhs=rhs_of(j),
                                             start=False,
                                             stop=(j == 8),
                                             skip_group_check=True)
                        yt = yp.tile([128, BL], F32, tag="yt", name=f"yt{m}")
                        nc.vector.scalar_tensor_tensor(yt[:, :],
                                                       xc_tiles[m][:, :],
                                                       rs_t[:, :], psum[m][:, :],
                                                       AluOpType.mult,
                                                       AluOpType.add)
                        nc.sync.dma_start(out=y[m * 128:(m + 1) * 128, :],
                                          in_=yt[:, :])

    nc.compile()
    return nc


_NC_CACHE = None


def kernel(x, coeffs, base_weight, grid_steps_log, grid_start, res_scale,
           _trace=False):
    global _NC_CACHE, LAST_PROFILE

    x = np.asarray(x, dtype=np.float32)
    coeffs = np.asarray(coeffs, dtype=np.float32)
    base_weight = np.asarray(base_weight, dtype=np.float32)
    grid_steps_log = np.asarray(grid_steps_log, dtype=np.float32)
    grid_start = np.asarray(grid_start, dtype=np.float32)
    res_scale = np.asarray(res_scale, dtype=np.float32)

    # ---- host-side layout prep (pure reshape/transpose/dtype) ----
    # weights, k-order j-major: k = j*IN_DIM + i ; block j=8 is base_weight.T
    wj = coeffs.reshape(OUT_DIM, IN_DIM, 8).transpose(2, 1, 0)    # [8, in, out]
    big_w = np.concatenate([wj, base_weight.T[None]], axis=0)     # [9, in, out]
    big_w = np.ascontiguousarray(big_w.reshape(9 * IN_DIM, OUT_DIM),
                                 dtype=np.float16)

    xT = np.ascontiguousarray(x.T)                                # [in, B]
    # grid params: partition = in-dim within chunk, free = (chunk, knot)
    gsl_r = np.ascontiguousarray(
        grid_steps_log.reshape(NCH, 128, NK - 1).transpose(1, 2, 0)
        .reshape(128, (NK - 1) * NCH))
    gst_r = np.ascontiguousarray(
        grid_start.reshape(NCH, 128).T)                           # [128, 8]
    rs_r = res_scale.reshape(1, 1)

    if _NC_CACHE is None:
        _NC_CACHE = _build_nc()
    nc = _NC_CACHE

    in_maps = []
    for c in range(N_CORES):
        in_maps.append({
            "xt": np.ascontiguousarray(xT[:, c * BL:(c + 1) * BL]),
            "w": big_w,
            "gsl": gsl_r,
            "gst": gst_r,
            "rs": rs_r,
        })

    res = run_bass_kernel_spmd(nc, in_maps, core_ids=list(range(N_CORES)),
                               trace=_trace)
    LAST_PROFILE = {
        "exec_time_ns": res.exec_time_ns,
        "mean_exec_time_ns": res.mean_exec_time_ns,
        "max_exec_time_core_id": res.max_exec_time_core_id,
        "profile_json": res.profile_json,
        "instructions_and_trace": res.instructions_and_trace,
    }

    out = np.concatenate([r["y"].T for r in res.results], axis=0)  # [B, out]
    return np.ascontiguousarray(out.astype(np.float32))


# revision 24
# speedup vs baseline: 1.0131x; 1.0044x over previous
"""BSpline KAN layer (grid_size=5, spline_order=3) on 8 Trainium2 NeuronCores.

Strategy (data-parallel over batch):
  - Each core gets B_local = 512 rows of x, replicated weights.
  - Layout on-chip: in-dim on partitions (8 chunks of 128), batch on free dim.
  - Grid -> knots/reciprocals computed on-device per in-chunk column ([128,1]
    per-partition scalars for tensor_scalar / activation scale+bias).
  - Degree-1 bases as hat functions: b1[j] = min(relu(up), relu(down)), with
    up/down computed on the Scalar (ACT) engine as Relu(x*scale+bias).
  - Degrees 2/3 via Cox-de Boor with l/r factors from fused tensor_scalar
    ((x - g[j]) * recip) at DVE 4x mode, and the products/sums as j-stacked
    wide tensor_tensor ops ([128, 9*512]) to amortize instruction overhead.
  - Spline contraction as matmul with k-order j-major: k = j*1024 + i, so the
    j-stacked basis tiles are directly the matmul rhs. silu(x) @ base_weight.T
    is folded in as a 9th "basis" with base_weight as its weight block.
  - All 8 PSUM banks accumulate the 8 out-chunks across the whole contraction;
    epilogue adds res_scale * x and stores y[out, batch] (host transposes).
Precision: fp16 bases/weights, fp32 accumulation (emulated L2 rel err ~5e-4).
"""

import numpy as np

import concourse.bass as bass
from concourse import bacc
import concourse.mybir as mybir
import concourse.tile as tile
from concourse.alu_op_type import AluOpType
from concourse.bass_utils import run_bass_kernel_spmd

F32 = mybir.dt.float32
F16 = mybir.dt.float16
AF = mybir.ActivationFunctionType

IN_DIM = 1024
OUT_DIM = 1024
BATCH = 4096
N_CORES = 8
BL = BATCH // N_CORES        # 512 batch rows per core
NCH = IN_DIM // 128          # 8 in-dim chunks
NK = 12                      # knots per dim
EPS = 1e-8

LAST_PROFILE = {}

# engine for the stacked adds of the recursion (offload DVE)
B2_ADD_ENGINE = "gpsimd"
B3_ADD_ENGINE = "gpsimd"


def _build_nc():
    nc = bacc.Bacc("TRN2", target_bir_lowering=False)

    xt = nc.dram_tensor("xt", [IN_DIM, BL], F32, kind="ExternalInput")
    w = nc.dram_tensor("w", [9 * IN_DIM, OUT_DIM], F16, kind="ExternalInput")
    gsl = nc.dram_tensor("gsl", [128, NCH * (NK - 1)], F32, kind="ExternalInput")
    gst = nc.dram_tensor("gst", [128, NCH], F32, kind="ExternalInput")
    rs = nc.dram_tensor("rs", [1, 1], F32, kind="ExternalInput")
    y = nc.dram_tensor("y", [OUT_DIM, BL], F32, kind="ExternalOutput")

    with tile.TileContext(nc) as tc:
        with (
            tc.tile_pool(name="const", bufs=1) as cp,
            tc.tile_pool(name="xres", bufs=1) as xp,
            tc.tile_pool(name="small", bufs=4) as sp,
            tc.tile_pool(name="updn", bufs=2) as bp1,
            tc.tile_pool(name="lr2", bufs=2) as bp2,
            tc.tile_pool(name="lr3", bufs=3) as bp3,
            tc.tile_pool(name="wts", bufs=12) as wp,
            tc.tile_pool(name="yout", bufs=4) as yp,
            tc.tile_pool(name="psum", bufs=1, space="PSUM") as pp,
        ):
            # ---------------- grid preparation (once) ----------------
            gslT = cp.tile([128, NK - 1, NCH], F32)
            nc.gpsimd.dma_start(out=gslT[:, :, :],
                                in_=gsl[:, :].rearrange("p (k c) -> p k c", c=NCH))
            g3 = cp.tile([128, NK, NCH], F32)
            nc.gpsimd.dma_start(out=g3[:, 0, :], in_=gst[:, :])

            # softplus(v) = relu(v) + ln(1 + exp(-|v|))   (no softplus table
            # in the ACT func sets; exp/ln are in natural_log_exp_and_others)
            st3 = cp.tile([128, NK - 1, NCH], F32)
            spa = cp.tile([128, NK - 1, NCH], F32)
            nc.scalar.activation(spa[:, :, :], gslT[:, :, :], AF.Abs)
            nc.scalar.activation(spa[:, :, :], spa[:, :, :], AF.Exp, scale=-1.0)
            nc.scalar.activation(spa[:, :, :], spa[:, :, :], AF.Ln, bias=1.0)
            nc.scalar.activation(st3[:, :, :], gslT[:, :, :], AF.Relu)
            nc.vector.tensor_tensor(st3[:, :, :], st3[:, :, :], spa[:, :, :],
                                    AluOpType.add)
            for k in range(1, NK):
                nc.vector.tensor_tensor(g3[:, k, :], g3[:, k - 1, :],
                                        st3[:, k - 1, :], AluOpType.add)

            def recips(d, n):
                dt = cp.tile([128, n, NCH], F32, tag=f"d{d}")
                nc.vector.tensor_tensor(dt[:, :, :], g3[:, d:NK, :],
                                        g3[:, 0:NK - d, :], AluOpType.subtract)
                nc.vector.tensor_scalar_add(dt[:, :, :], dt[:, :, :], EPS)
                r = cp.tile([128, n, NCH], F32, tag=f"r{d}")
                nc.vector.reciprocal(r[:, :, :], dt[:, :, :])
                nr = cp.tile([128, n, NCH], F32, tag=f"nr{d}")
                nc.vector.tensor_scalar_mul(nr[:, :, :], r[:, :, :], -1.0)
                return r, nr

            R1, NR1 = recips(1, NK - 1)   # [128,8,11]
            R2, NR2 = recips(2, NK - 2)   # [128,8,10]
            R3, NR3 = recips(3, NK - 3)   # [128,8,9]

            # biases for the ACT hat ops
            BU = cp.tile([128, 10, NCH], F32)   # -g[j]*R1[j]
            nc.vector.scalar_tensor_tensor(BU[:, :, :], g3[:, 0:10, :], -1.0,
                                           R1[:, 0:10, :],
                                           AluOpType.mult, AluOpType.mult)
            BD = cp.tile([128, 10, NCH], F32)   # g[j+2]*R1[j+1]
            nc.vector.tensor_tensor(BD[:, :, :], g3[:, 2:12, :],
                                    R1[:, 1:11, :], AluOpType.mult)

            # biases for the ACT degree-3 factor ops
            BL3 = cp.tile([128, 8, NCH], F32)   # -g[j]*R3[j]
            nc.vector.scalar_tensor_tensor(BL3[:, :, :], g3[:, 0:8, :], -1.0,
                                           R3[:, 0:8, :],
                                           AluOpType.mult, AluOpType.mult)
            BR3 = cp.tile([128, 8, NCH], F32)   # g[j+4]*R3[j+1]
            nc.vector.tensor_tensor(BR3[:, :, :], g3[:, 4:12, :],
                                    R3[:, 1:9, :], AluOpType.mult)

            rs_t = cp.tile([128, 1], F32)
            nc.gpsimd.dma_start(out=rs_t[:, :], in_=rs[:].to_broadcast((128, 1)))

            # PSUM accumulators: one bank per out-chunk
            psum = [pp.tile([128, BL], F32, tag=f"ps{m}", name=f"ps{m}")
                    for m in range(NCH)]

            xc_tiles = []
            # ---------------- main loop over in-chunks ----------------
            for c in range(NCH):
                xc = xp.tile([128, BL], F32, tag=f"xc{c}")
                nc.sync.dma_start(out=xc[:, :], in_=xt[c * 128:(c + 1) * 128, :])
                xc_tiles.append(xc)

                x16 = sp.tile([128, BL], F16, tag="x16")
                nc.vector.tensor_scalar(x16[:, :], xc[:, :], 1.0,
                                        None, AluOpType.mult)
                # degree-1 hats on ACT; the independent DVE tensor_scalar
                # factor ops are emitted FIRST so the in-order DVE queue has
                # ready work while the 20 ACT hat ops complete (the min below
                # blocks the DVE FIFO until the hats land)
                UP = bp1.tile([128, 10, BL], F16, tag="up")
                DN = bp1.tile([128, 10, BL], F16, tag="dn")
                for j in range(10):
                    nc.scalar.activation(UP[:, j, :], x16[:, :], AF.Relu,
                                         bias=BU[:, j, c:c+1], scale=R1[:, j, c:c+1])
                    nc.scalar.activation(DN[:, j, :], x16[:, :], AF.Relu,
                                         bias=BD[:, j, c:c+1], scale=NR1[:, j+1, c:c+1])

                L2 = bp2.tile([128, 10, BL], F16, tag="l2")
                R2t = bp2.tile([128, 9, BL], F16, tag="r2")
                L3 = bp3.tile([128, 8, BL], F16, tag="l3")
                R3t = bp3.tile([128, 8, BL], F16, tag="r3")
                for j in range(10):
                    nc.vector.tensor_scalar(L2[:, j, :], x16[:, :],
                                            g3[:, j, c:c+1], R2[:, j, c:c+1],
                                            AluOpType.subtract, AluOpType.mult)
                # r2[j] = 1 - l2[j+1] (same denominator, exact to ref's eps);
                # one wide immediate-scalar op replaces 9 pointer-scalar ops
                nc.vector.tensor_scalar(R2t[:, :, :], L2[:, 1:10, :], -1.0,
                                        1.0, AluOpType.mult, AluOpType.add)
                for j in range(3):
                    nc.vector.tensor_scalar(L3[:, j, :], x16[:, :],
                                            g3[:, j, c:c+1], R3[:, j, c:c+1],
                                            AluOpType.subtract, AluOpType.mult)
                for j in range(8):
                    if j >= 3:
                        nc.scalar.activation(L3[:, j, :], x16[:, :], AF.Identity,
                                             bias=BL3[:, j, c:c+1],
                                             scale=R3[:, j, c:c+1])
                    nc.scalar.activation(R3t[:, j, :], x16[:, :], AF.Identity,
                                         bias=BR3[:, j, c:c+1],
                                         scale=NR3[:, j+1, c:c+1])
                sil = sp.tile([128, BL], F16, tag="sil")
                nc.scalar.activation(sil[:, :], x16[:, :], AF.Silu)

                nc.vector.tensor_tensor(UP[:, :, :], UP[:, :, :], DN[:, :, :],
                                        AluOpType.min)   # b1 := UP
                nc.vector.tensor_tensor(L2[:, 0:9, :], L2[:, 0:9, :],
                                        UP[:, 0:9, :], AluOpType.mult)
                nc.vector.tensor_tensor(R2t[:, :, :], R2t[:, :, :],
                                        UP[:, 1:10, :], AluOpType.mult)
                nc.vector.tensor_tensor(L2[:, 0:9, :], L2[:, 0:9, :],
                                        R2t[:, :, :], AluOpType.add)  # b2
                nc.vector.tensor_tensor(L3[:, :, :], L3[:, :, :],
                                        L2[:, 0:8, :], AluOpType.mult)
                nc.vector.tensor_tensor(R3t[:, :, :], R3t[:, :, :],
                                        L2[:, 1:9, :], AluOpType.mult)
                nc.vector.tensor_tensor(L3[:, :, :], L3[:, :, :],
                                        R3t[:, :, :], AluOpType.add)  # b3

                # matmuls: 9 weight blocks (8 spline j's + silu/base_weight)
                wts = []
                for j in range(9):
                    kc = j * NCH + c
                    wt = wp.tile([128, OUT_DIM], F16, tag="wt", name=f"wt{c}_{j}")
                    nc.sync.dma_start(out=wt[:, :],
                                      in_=w[kc * 128:(kc + 1) * 128, :])
                    wts.append(wt)

                def rhs_of(j):
                    return L3[:, j, :] if j < 8 else sil[:, :]

                if c < NCH - 1:
                    for j in range(9):
                        for m in range(NCH):
                            nc.tensor.matmul(psum[m][:, :],
                                             lhsT=wts[j][:, m * 128:(m + 1) * 128],
                                             rhs=rhs_of(j),
                                             start=(c == 0 and j == 0),
                                             stop=False,
                                             skip_group_check=True)
                else:
                    # last chunk: m-outer so each PSUM bank finishes early and
                    # its epilogue overlaps the remaining matmuls
                    for m in range(NCH):
                        for j in range(9):
                            nc.tensor.matmul(psum[m][:, :],
                                             lhsT=wts[j][:, m * 128:(m + 1) * 128],
                                             rhs=rhs_of(j),
                                             start=False,
                                             stop=(j == 8),
                                             skip_group_check=True)
                        yt = yp.tile([128, BL], F32, tag="yt", name=f"yt{m}")
                        nc.vector.scalar_tensor_tensor(yt[:, :],
                                                       xc_tiles[m][:, :],
                                                       rs_t[:, :], psum[m][:, :],
                                                       AluOpType.mult,
                                                       AluOpType.add)
                        nc.sync.dma_start(out=y[m * 128:(m + 1) * 128, :],
                                          in_=yt[:, :])

    nc.compile()
    return nc


_NC_CACHE = None


def kernel(x, coeffs, base_weight, grid_steps_log, grid_start, res_scale,
           _trace=False):
    global _NC_CACHE, LAST_PROFILE

    x = np.asarray(x, dtype=np.float32)
    coeffs = np.asarray(coeffs, dtype=np.float32)
    base_weight = np.asarray(base_weight, dtype=np.float32)
    grid_steps_log = np.asarray(grid_steps_log, dtype=np.float32)
    grid_start = np.asarray(grid_start, dtype=np.float32)
    res_scale = np.asarray(res_scale, dtype=np.float32)

    # ---- host-side layout prep (pure reshape/transpose/dtype) ----
    # weights, k-order j-major: k = j*IN_DIM + i ; block j=8 is base_weight.T
    wj = coeffs.reshape(OUT_DIM, IN_DIM, 8).transpose(2, 1, 0)    # [8, in, out]
    big_w = np.concatenate([wj, base_weight.T[None]], axis=0)     # [9, in, out]
    big_w = np.ascontiguousarray(big_w.reshape(9 * IN_DIM, OUT_DIM),
                                 dtype=np.float16)

    xT = np.ascontiguousarray(x.T)                                # [in, B]
    # grid params: partition = in-dim within chunk, free = (chunk, knot)
    gsl_r = np.ascontiguousarray(
        grid_steps_log.reshape(NCH, 128, NK - 1).transpose(1, 2, 0)
        .reshape(128, (NK - 1) * NCH))
    gst_r = np.ascontiguousarray(
        grid_start.reshape(NCH, 128).T)                           # [128, 8]
    rs_r = res_scale.reshape(1, 1)

    if _NC_CACHE is None:
        _NC_CACHE = _build_nc()
    nc = _NC_CACHE

    in_maps = []
    for c in range(N_CORES):
        in_maps.append({
            "xt": np.ascontiguousarray(xT[:, c * BL:(c + 1) * BL]),
            "w": big_w,
            "gsl": gsl_r,
            "gst": gst_r,
            "rs": rs_r,
        })

    res = run_bass_kernel_spmd(nc, in_maps, core_ids=list(range(N_CORES)),
                               trace=_trace)
    LAST_PROFILE = {
        "exec_time_ns": res.exec_time_ns,
        "mean_exec_time_ns": res.mean_exec_time_ns,
        "max_exec_time_core_id": res.max_exec_time_core_id,
        "profile_json": res.profile_json,
        "instructions_and_trace": res.instructions_and_trace,
    }

    out = np.concatenate([r["y"].T for r in res.results], axis=0)  # [B, out]
    return np.ascontiguousarray(out.astype(np.float32))
